# revision 5
# baseline (speedup 1.0000x reference)
"""Multi-head self-attention Trainium2 kernel.

Problem: B=2, N=2048, D=1024, H=16 heads (HD=64), fp32 I/O.

Sharding (8 cores): core c handles batch b = c//4 and the 4-head group
g = c%4 (data parallel on B, tensor parallel on heads).  Each core:
  1. QKV projection for its 768 columns (q cols pre-scaled by HD^-0.5,
     bias folded in as a K=1 matmul against a ones row),
     producing qT/kT channel-major [64, 2048] per head and V row-major
     [2048, 64] per head augmented with a ones column.
  2. Transposed attention per head: S^T[m, n] = kT.T-free matmul (K=64
     contraction over head dim), exp on ScalarE (no max subtraction --
     logits are O(1) here), PV matmul contracting over m with the ones
     column yielding the softmax denominator as output row 64.
  3. Normalization: reciprocal of the denominator row, broadcast across
     64 partitions via a K=1 matmul, multiply -> out^T channel-major.
  4. Output projection against its 256 rows of w_proj -> fp32 partial.
Host sums the 4 partials per batch and adds b_proj.
"""

import numpy as np
import ml_dtypes

B, N, D, H = 2, 2048, 1024, 16
HD = D // H  # 64
SCALE = HD ** -0.5
NCORES = 8
HPC = H // 4  # heads per core
CPC = HPC * HD  # channels per core = 256
P = 128
DT = D // P  # 8 contraction tiles
NT = N // P  # 16 sequence tiles

_CACHE = {}


def build_nc():
    import concourse.tile as tile
    from concourse import bacc, mybir

    nc = bacc.Bacc("TRN2", target_bir_lowering=False, debug=False,
                   num_devices=NCORES)
    bf16 = mybir.dt.bfloat16
    f32 = mybir.dt.float32
    xt = nc.dram_tensor("xt", [D, N], bf16, kind="ExternalInput").ap()
    w = nc.dram_tensor("w", [D + 1, 3 * CPC], bf16, kind="ExternalInput").ap()
    wp = nc.dram_tensor("wp", [CPC, D], bf16, kind="ExternalInput").ap()
    y = nc.dram_tensor("y", [N, D], f32, kind="ExternalOutput").ap()

    with tile.TileContext(nc) as tc:
        _mha_tile_kernel(tc, y, xt, w, wp)
    nc.compile()
    return nc


def _mha_tile_kernel(tc, y, xt, w, wp):
    from contextlib import ExitStack
    import concourse.tile as tile  # noqa: F401
    from concourse import mybir

    nc = tc.nc
    bf16 = mybir.dt.bfloat16
    f32 = mybir.dt.float32
    EXP = mybir.ActivationFunctionType.Exp
    HN = N // 2  # 1024

    with ExitStack() as ctx:
        consts = ctx.enter_context(tc.tile_pool(name="consts", bufs=1))
        work = ctx.enter_context(tc.tile_pool(name="work", bufs=1))
        ebpool = ctx.enter_context(tc.tile_pool(name="eb", bufs=3))
        ypool = ctx.enter_context(tc.tile_pool(name="yp", bufs=3))
        rpool = ctx.enter_context(tc.tile_pool(name="rp", bufs=4))
        ps_big = ctx.enter_context(
            tc.tile_pool(name="ps_big", bufs=2, space="PSUM"))
        ps_pv = ctx.enter_context(
            tc.tile_pool(name="ps_pv", bufs=2, space="PSUM"))
        ps_sm = ctx.enter_context(
            tc.tile_pool(name="ps_sm", bufs=2, space="PSUM"))

        # ---- load weights/activations ----
        xt_sb = work.tile([P, DT, N], bf16, tag="xt")
        for kt in range(DT):
            nc.sync.dma_start(xt_sb[:, kt], xt[kt * P:(kt + 1) * P, :])
        w_sb = work.tile([P, DT, 3 * CPC], bf16, tag="w")
        for kt in range(DT):
            nc.sync.dma_start(w_sb[:, kt], w[kt * P:(kt + 1) * P, :])
        wb_sb = work.tile([1, 3 * CPC], bf16, tag="wb")
        nc.sync.dma_start(wb_sb, w[D:D + 1, :])
        wp_sb = work.tile([P, 2, D], bf16, tag="wp")
        for ct in range(2):
            nc.sync.dma_start(wp_sb[:, ct], wp[ct * P:(ct + 1) * P, :])
        ones_sb = consts.tile([1, N], bf16, tag="ones")
        nc.vector.memset(ones_sb, 1.0)
        onesf_sb = consts.tile([1, HD], f32, tag="onesf")
        nc.vector.memset(onesf_sb, 1.0)

        # ---- qT / kT channel-major [128, 4 ctiles, N] ----
        # ct 0,1 = q heads (0,1),(2,3); ct 2,3 = k heads (0,1),(2,3)
        qk_sb = work.tile([P, 4, N], bf16, tag="qk")
        for ct in range(4):
            wcol = ct * P
            for half in range(2):
                n0 = half * HN
                ps = ps_big.tile([P, HN], f32, tag="big")
                for dt in range(DT):
                    for j in range(2):
                        nc.tensor.matmul(
                            ps[:, j * 512:(j + 1) * 512],
                            lhsT=w_sb[:, dt, wcol:wcol + P],
                            rhs=xt_sb[:, dt, n0 + j * 512:n0 + (j + 1) * 512],
                            start=(dt == 0), stop=False)
                for j in range(2):
                    nc.tensor.matmul(
                        ps[:, j * 512:(j + 1) * 512],
                        lhsT=wb_sb[:, wcol:wcol + P],
                        rhs=ones_sb[:, n0 + j * 512:n0 + (j + 1) * 512],
                        start=False, stop=True)
                nc.vector.tensor_copy(out=qk_sb[:, ct, n0:n0 + HN], in_=ps)

        # ---- V row-major, augmented with ones column ----
        vaug_sb = work.tile([P, NT, HPC, HD + 1], bf16, tag="vaug")
        nc.vector.memset(vaug_sb[:, :, :, HD:HD + 1], 1.0)
        for mt in range(NT):
            ps = ps_sm.tile([P, CPC], f32, tag="sm")
            for dt in range(DT):
                nc.tensor.matmul(
                    ps, lhsT=xt_sb[:, dt, mt * P:(mt + 1) * P],
                    rhs=w_sb[:, dt, 2 * CPC:3 * CPC],
                    start=(dt == 0), stop=False)
            nc.tensor.matmul(
                ps, lhsT=ones_sb[:, mt * P:(mt + 1) * P],
                rhs=wb_sb[:, 2 * CPC:3 * CPC], start=False, stop=True)
            for h in range(HPC):
                nc.vector.tensor_copy(
                    out=vaug_sb[:, mt, h, 0:HD],
                    in_=ps[:, h * HD:(h + 1) * HD])

        # ---- attention per head ----
        outT_sb = work.tile([P, 2, N], bf16, tag="outT")
        for h in range(HPC):
            bp = (h % 2) * HD
            qT = qk_sb[bp:bp + HD, h // 2, :]
            kT = qk_sb[bp:bp + HD, 2 + h // 2, :]
            for half in range(2):
                n0 = half * HN
                pv = [ps_pv.tile([HD + 1, 512], f32, tag="pv", name=f"pv{j}")
                      for j in range(2)]
                for mt in range(NT):
                    ps_s = ps_big.tile([P, HN], f32, tag="big")
                    for j in range(2):
                        nc.tensor.matmul(
                            ps_s[:, j * 512:(j + 1) * 512],
                            lhsT=kT[:, mt * P:(mt + 1) * P],
                            rhs=qT[:, n0 + j * 512:n0 + (j + 1) * 512],
                            start=True, stop=True)
                    eb = ebpool.tile([P, HN], bf16, tag="eb")
                    nc.scalar.activation(out=eb, in_=ps_s, func=EXP)
                    for j in range(2):
                        nc.tensor.matmul(
                            pv[j], lhsT=vaug_sb[:, mt, h, :],
                            rhs=eb[:, j * 512:(j + 1) * 512],
                            start=(mt == 0), stop=(mt == NT - 1))
                for j in range(2):
                    rec = rpool.tile([1, 512], f32, tag="rec")
                    nc.vector.reciprocal(out=rec, in_=pv[j][HD:HD + 1, :])
                    bc = ps_sm.tile([HD, 512], f32, tag="sm")
                    nc.tensor.matmul(bc, lhsT=onesf_sb, rhs=rec,
                                     start=True, stop=True)
                    bcs = rpool.tile([HD, 512], f32, tag="bcs")
                    nc.vector.tensor_copy(out=bcs, in_=bc)
                    nc.vector.tensor_mul(
                        out=outT_sb[bp:bp + HD, h // 2,
                                    n0 + j * 512:n0 + (j + 1) * 512],
                        in0=pv[j][0:HD, :], in1=bcs)

        # ---- output projection (fp32 partial) ----
        for nt in range(NT):
            yt = ypool.tile([P, D], f32, tag="y")
            for ec in range(2):
                ps = ps_sm.tile([P, 512], f32, tag="sm")
                for ct in range(2):
                    nc.tensor.matmul(
                        ps, lhsT=outT_sb[:, ct, nt * P:(nt + 1) * P],
                        rhs=wp_sb[:, ct, ec * 512:(ec + 1) * 512],
                        start=(ct == 0), stop=(ct == 1))
                nc.vector.tensor_copy(out=yt[:, ec * 512:(ec + 1) * 512],
                                      in_=ps)
            nc.sync.dma_start(y[nt * P:(nt + 1) * P, :], yt)


def make_in_maps(x, w_qkv, b_qkv, w_proj):
    """Build the 8 per-core input dicts (host-side sharding)."""
    bf = ml_dtypes.bfloat16
    x = np.asarray(x, np.float32)
    w_qkv = np.asarray(w_qkv, np.float32)
    b_qkv = np.asarray(b_qkv, np.float32)
    w_proj = np.asarray(w_proj, np.float32)

    xts = [np.ascontiguousarray(x[b].T).astype(bf) for b in range(B)]
    w_augs = []
    wps = []
    for g in range(4):
        c0 = g * CPC
        wq = w_qkv[:, c0:c0 + CPC] * SCALE
        wk = w_qkv[:, D + c0:D + c0 + CPC]
        wv = w_qkv[:, 2 * D + c0:2 * D + c0 + CPC]
        bq = b_qkv[c0:c0 + CPC] * SCALE
        bk = b_qkv[D + c0:D + c0 + CPC]
        bv = b_qkv[2 * D + c0:2 * D + c0 + CPC]
        w_slice = np.concatenate([wq, wk, wv], axis=1)
        b_slice = np.concatenate([bq, bk, bv])
        w_aug = np.vstack([w_slice, b_slice[None, :]]).astype(bf)
        w_augs.append(np.ascontiguousarray(w_aug))
        wps.append(np.ascontiguousarray(w_proj[c0:c0 + CPC, :]).astype(bf))

    in_maps = []
    for core in range(NCORES):
        b, g = core // 4, core % 4
        in_maps.append({"xt": xts[b], "w": w_augs[g], "wp": wps[g]})
    return in_maps


def core_reference(in_map):
    """Numpy reference for ONE core's shard (for CoreSim verification)."""
    xt = np.asarray(in_map["xt"], np.float32)  # [D, N]
    w = np.asarray(in_map["w"], np.float32)    # [D+1, 768]
    wp = np.asarray(in_map["wp"], np.float32)  # [256, D]
    qkv = xt.T @ w[:D] + w[D]                  # [N, 768]
    out = np.zeros((N, CPC), np.float32)
    for h in range(HPC):
        q = qkv[:, h * HD:(h + 1) * HD]
        k = qkv[:, CPC + h * HD:CPC + (h + 1) * HD]
        v = qkv[:, 2 * CPC + h * HD:2 * CPC + (h + 1) * HD]
        s = q @ k.T  # scale already folded into wq
        p = np.exp(s - s.max(axis=-1, keepdims=True))
        p /= p.sum(axis=-1, keepdims=True)
        out[:, h * HD:(h + 1) * HD] = p @ v
    return out @ wp  # [N, D] partial


def kernel(x, w_qkv, b_qkv, w_proj, b_proj):
    from concourse.bass_utils import run_bass_kernel_spmd

    in_maps = make_in_maps(x, w_qkv, b_qkv, w_proj)
    if "nc" not in _CACHE:
        _CACHE["nc"] = build_nc()
    res = run_bass_kernel_spmd(_CACHE["nc"], in_maps,
                               core_ids=list(range(NCORES)))
    outs = [r["y"] for r in res.results]
    y = np.empty((B, N, D), np.float32)
    for b in range(B):
        y[b] = outs[4 * b] + outs[4 * b + 1] + outs[4 * b + 2] + outs[4 * b + 3]
    y += np.asarray(b_proj, np.float32)
    return y


# revision 7
# speedup vs baseline: 1.2159x; 1.2159x over previous
"""Multi-head self-attention Trainium2 kernel.

Problem: B=2, N=2048, D=1024, H=16 heads (HD=64), fp32 I/O.

Sharding (8 cores): core c handles batch b = c//4 and the 4-head group
g = c%4 (data parallel on B, tensor parallel on heads).  Each core:
  1. QKV projection for its 768 columns (q cols pre-scaled by HD^-0.5,
     bias folded in as a K=1 matmul against a ones row), producing
     qT/kT channel-major and V row-major augmented with a ones column.
  2. Transposed attention, two heads packed per pass (head A in PE rows
     0-63, head B in rows 64-127 -> concurrent row-group matmuls):
     S^T[m, n] scores in PSUM, one exp per m-tile on ScalarE (no max
     subtraction -- logits are O(1) here), PV matmul contracting over m
     with the ones column yielding the softmax denominator as row 64.
  3. Normalization: fast-approx reciprocal of the denominator row,
     broadcast across 64 partitions via a K=1 matmul, multiply.
  4. Output projection against its 256 rows of w_proj -> fp32 partial.
Host sums the 4 partials per batch and adds b_proj.

Emission order is chosen to keep the PE array dense (HAM stays at
K=8/8): QKV matmul groups are injected one-per-iteration into the
first attention pair's ACT-bound window.
"""

import numpy as np
import ml_dtypes

B, N, D, H = 2, 2048, 1024, 16
HD = D // H  # 64
SCALE = HD ** -0.5
NCORES = 8
HPC = H // 4  # heads per core
CPC = HPC * HD  # channels per core = 256
P = 128
DT = D // P  # 8 contraction tiles
NT = N // P  # 16 sequence tiles

_CACHE = {}


def build_nc():
    import concourse.tile as tile
    from concourse import bacc, mybir

    nc = bacc.Bacc("TRN2", target_bir_lowering=False, debug=False,
                   num_devices=NCORES)
    bf16 = mybir.dt.bfloat16
    f32 = mybir.dt.float32
    xt = nc.dram_tensor("xt", [D, N], bf16, kind="ExternalInput").ap()
    w = nc.dram_tensor("w", [D + 1, 3 * CPC], bf16, kind="ExternalInput").ap()
    wp = nc.dram_tensor("wp", [CPC, D], bf16, kind="ExternalInput").ap()
    y = nc.dram_tensor("y", [N, D], f32, kind="ExternalOutput").ap()

    with tile.TileContext(nc) as tc:
        _mha_tile_kernel(tc, y, xt, w, wp)
    nc.compile()
    return nc


def _mha_tile_kernel(tc, y, xt, w, wp):
    from contextlib import ExitStack
    from concourse import mybir

    nc = tc.nc
    bf16 = mybir.dt.bfloat16
    f32 = mybir.dt.float32
    EXP = mybir.ActivationFunctionType.Exp

    with ExitStack() as ctx:
        consts = ctx.enter_context(tc.tile_pool(name="consts", bufs=1))
        work = ctx.enter_context(tc.tile_pool(name="work", bufs=1))
        ebpool = ctx.enter_context(tc.tile_pool(name="eb", bufs=3))
        ypool = ctx.enter_context(tc.tile_pool(name="yp", bufs=3))
        rpool = ctx.enter_context(tc.tile_pool(name="rp", bufs=2))
        ps_sc = ctx.enter_context(
            tc.tile_pool(name="ps_sc", bufs=2, space="PSUM"))   # 2x2 banks
        ps_pv = ctx.enter_context(
            tc.tile_pool(name="ps_pv", bufs=2, space="PSUM"))   # 2x1 banks
        ps_sm = ctx.enter_context(
            tc.tile_pool(name="ps_sm", bufs=1, space="PSUM"))   # 1x2 banks

        # ---- loads ----
        xt_sb = work.tile([P, DT, N], bf16, tag="xt")
        for kt in range(DT):
            nc.sync.dma_start(xt_sb[:, kt], xt[kt * P:(kt + 1) * P, :])
        w_sb = work.tile([P, DT, 3 * CPC], bf16, tag="w")
        for kt in range(DT):
            nc.sync.dma_start(w_sb[:, kt], w[kt * P:(kt + 1) * P, :])
        wb_sb = work.tile([1, 3 * CPC], bf16, tag="wb")
        nc.sync.dma_start(wb_sb, w[D:D + 1, :])
        wp_sb = work.tile([P, 2, D], bf16, tag="wp")
        for ct in range(2):
            nc.sync.dma_start(wp_sb[:, ct], wp[ct * P:(ct + 1) * P, :])
        ones_sb = consts.tile([1, N], bf16, tag="ones")
        nc.vector.memset(ones_sb, 1.0)

        qk_sb = work.tile([P, 4, N], bf16, tag="qk")
        vaug_sb = work.tile([P, NT, HPC, HD + 1], bf16, tag="vaug")
        nc.vector.memset(vaug_sb[:, :, :, HD:HD + 1], 1.0)
        outT_sb = work.tile([P, 2, N], bf16, tag="outT")

        # ---- emission helpers ----
        def emit_qk_group(ct, half, engine):
            """qT/kT channel-major: psum[c 128, n 1024] accumulated over
            d; bias via K=1 matmul; copy to qk_sb as bf16."""
            wcol = ct * P
            n0 = half * 1024
            ps = ps_sm.tile([P, 1024], f32, tag="sm", name=f"qk{ct}{half}")
            for dt in range(DT):
                for j in range(2):
                    nc.tensor.matmul(
                        ps[:, j * 512:(j + 1) * 512],
                        lhsT=w_sb[:, dt, wcol:wcol + P],
                        rhs=xt_sb[:, dt, n0 + j * 512:n0 + (j + 1) * 512],
                        start=(dt == 0), stop=False)
            for j in range(2):
                nc.tensor.matmul(
                    ps[:, j * 512:(j + 1) * 512],
                    lhsT=wb_sb[:, wcol:wcol + P],
                    rhs=ones_sb[:, n0 + j * 512:n0 + (j + 1) * 512],
                    start=False, stop=True)
            if engine is nc.scalar:
                nc.scalar.copy(out=qk_sb[:, ct, n0:n0 + 1024], in_=ps)
            else:
                engine.tensor_copy(out=qk_sb[:, ct, n0:n0 + 1024], in_=ps)

        def emit_v_group(mt, pair):
            """V row-major for head pair: psum[m 128, c 128] over d,
            bias via K=1, then per-head copies into vaug."""
            c0 = 2 * CPC + pair * P
            ps = ps_sm.tile([P, P], f32, tag="sm", name=f"v{pair}_{mt}")
            for dt in range(DT):
                nc.tensor.matmul(
                    ps, lhsT=xt_sb[:, dt, mt * P:(mt + 1) * P],
                    rhs=w_sb[:, dt, c0:c0 + P],
                    start=(dt == 0), stop=False)
            nc.tensor.matmul(
                ps, lhsT=ones_sb[:, mt * P:(mt + 1) * P],
                rhs=wb_sb[:, c0:c0 + P], start=False, stop=True)
            for i in range(2):
                nc.vector.tensor_copy(
                    out=vaug_sb[:, mt, 2 * pair + i, 0:HD],
                    in_=ps[:, i * HD:(i + 1) * HD])

        # ---- prologue: just enough for attention pair 0 to start ----
        emit_qk_group(2, 0, nc.scalar)   # kT heads 0,1 (all m needed)
        emit_qk_group(2, 1, nc.scalar)
        emit_qk_group(0, 0, nc.scalar)   # qT heads 0,1 cols 0:1024
        for mt in range(4):
            emit_v_group(mt, 0)

        # work to interleave into attention pair 0 (one group/iteration)
        inject = []
        inject += [lambda mt=mt: emit_v_group(mt, 0) for mt in range(4, NT)]
        inject.append(lambda: emit_qk_group(0, 1, nc.vector))
        inject.append(lambda: emit_qk_group(1, 0, nc.vector))
        inject.append(lambda: emit_qk_group(1, 1, nc.vector))
        inject.append(lambda: emit_qk_group(3, 0, nc.vector))
        inject.append(lambda: emit_qk_group(3, 1, nc.vector))
        inject += [lambda mt=mt: emit_v_group(mt, 1) for mt in range(NT)]

        # ---- attention: heads packed in pairs (rows 0-63 / 64-127) ----
        for pair in range(2):
            for q in range(4):
                n0 = q * 512
                pv = [ps_pv.tile([HD + 1, 512], f32, tag="pv",
                                 name=f"pv{pair}{q}{i}") for i in range(2)]
                for mt in range(NT):
                    if pair == 0 and inject:
                        inject.pop(0)()
                    ps = ps_sc.tile([P, 1024], f32, tag="sc")
                    for i in range(2):
                        bp = i * HD
                        nc.tensor.matmul(
                            ps[:, i * 512:(i + 1) * 512],
                            lhsT=qk_sb[bp:bp + HD, 2 + pair,
                                       mt * P:(mt + 1) * P],
                            rhs=qk_sb[bp:bp + HD, pair, n0:n0 + 512],
                            start=True, stop=True)
                    eb = ebpool.tile([P, 1024], bf16, tag="eb")
                    nc.scalar.activation(out=eb, in_=ps, func=EXP)
                    for i in range(2):
                        nc.tensor.matmul(
                            pv[i], lhsT=vaug_sb[:, mt, 2 * pair + i, :],
                            rhs=eb[:, i * 512:(i + 1) * 512],
                            start=(mt == 0), stop=(mt == NT - 1))
                for i in range(2):
                    bp = i * HD
                    dcp = rpool.tile([1, 512], f32, tag="dcp")
                    nc.vector.tensor_copy(out=dcp, in_=pv[i][HD:HD + 1, :])
                    rec = rpool.tile([1, 512], f32, tag="rec")
                    nc.vector.reciprocal_approx_fast(out=rec, in_=dcp)
                    rbf = rpool.tile([1, 512], bf16, tag="rbf")
                    nc.vector.tensor_copy(out=rbf, in_=rec)
                    bc = ps_sm.tile([HD, 512], f32, tag="sm",
                                    name=f"bc{pair}{q}{i}")
                    nc.tensor.matmul(bc, lhsT=ones_sb[:, 0:HD], rhs=rbf,
                                     start=True, stop=True)
                    bcs = rpool.tile([HD, 512], f32, tag="bcs")
                    nc.vector.tensor_copy(out=bcs, in_=bc)
                    nc.vector.tensor_mul(
                        out=outT_sb[bp:bp + HD, pair, n0:n0 + 512],
                        in0=pv[i][0:HD, :], in1=bcs)

        # ---- output projection (fp32 partial) ----
        for nt in range(NT):
            yt = ypool.tile([P, D], f32, tag="y")
            for ec in range(2):
                ps = ps_sm.tile([P, 512], f32, tag="sm", name=f"pj{nt}{ec}")
                for ct in range(2):
                    nc.tensor.matmul(
                        ps, lhsT=outT_sb[:, ct, nt * P:(nt + 1) * P],
                        rhs=wp_sb[:, ct, ec * 512:(ec + 1) * 512],
                        start=(ct == 0), stop=(ct == 1))
                nc.vector.tensor_copy(out=yt[:, ec * 512:(ec + 1) * 512],
                                      in_=ps)
            nc.sync.dma_start(y[nt * P:(nt + 1) * P, :], yt)


def make_in_maps(x, w_qkv, b_qkv, w_proj):
    """Build the 8 per-core input dicts (host-side sharding)."""
    bf = ml_dtypes.bfloat16
    x = np.asarray(x, np.float32)
    w_qkv = np.asarray(w_qkv, np.float32)
    b_qkv = np.asarray(b_qkv, np.float32)
    w_proj = np.asarray(w_proj, np.float32)

    xts = [np.ascontiguousarray(x[b].T).astype(bf) for b in range(B)]
    w_augs = []
    wps = []
    for g in range(4):
        c0 = g * CPC
        wq = w_qkv[:, c0:c0 + CPC] * SCALE
        wk = w_qkv[:, D + c0:D + c0 + CPC]
        wv = w_qkv[:, 2 * D + c0:2 * D + c0 + CPC]
        bq = b_qkv[c0:c0 + CPC] * SCALE
        bk = b_qkv[D + c0:D + c0 + CPC]
        bv = b_qkv[2 * D + c0:2 * D + c0 + CPC]
        w_slice = np.concatenate([wq, wk, wv], axis=1)
        b_slice = np.concatenate([bq, bk, bv])
        w_aug = np.vstack([w_slice, b_slice[None, :]]).astype(bf)
        w_augs.append(np.ascontiguousarray(w_aug))
        wps.append(np.ascontiguousarray(w_proj[c0:c0 + CPC, :]).astype(bf))

    in_maps = []
    for core in range(NCORES):
        b, g = core // 4, core % 4
        in_maps.append({"xt": xts[b], "w": w_augs[g], "wp": wps[g]})
    return in_maps


def core_reference(in_map):
    """Numpy reference for ONE core's shard (for CoreSim verification)."""
    xt = np.asarray(in_map["xt"], np.float32)  # [D, N]
    w = np.asarray(in_map["w"], np.float32)    # [D+1, 768]
    wp = np.asarray(in_map["wp"], np.float32)  # [256, D]
    qkv = xt.T @ w[:D] + w[D]                  # [N, 768]
    out = np.zeros((N, CPC), np.float32)
    for h in range(HPC):
        q = qkv[:, h * HD:(h + 1) * HD]
        k = qkv[:, CPC + h * HD:CPC + (h + 1) * HD]
        v = qkv[:, 2 * CPC + h * HD:2 * CPC + (h + 1) * HD]
        s = q @ k.T  # scale already folded into wq
        p = np.exp(s - s.max(axis=-1, keepdims=True))
        p /= p.sum(axis=-1, keepdims=True)
        out[:, h * HD:(h + 1) * HD] = p @ v
    return out @ wp  # [N, D] partial


def kernel(x, w_qkv, b_qkv, w_proj, b_proj):
    from concourse.bass_utils import run_bass_kernel_spmd

    in_maps = make_in_maps(x, w_qkv, b_qkv, w_proj)
    if "nc" not in _CACHE:
        _CACHE["nc"] = build_nc()
    res = run_bass_kernel_spmd(_CACHE["nc"], in_maps,
                               core_ids=list(range(NCORES)))
    outs = [r["y"] for r in res.results]
    y = np.empty((B, N, D), np.float32)
    for b in range(B):
        y[b] = outs[4 * b] + outs[4 * b + 1] + outs[4 * b + 2] + outs[4 * b + 3]
    y += np.asarray(b_proj, np.float32)
    return y


# revision 9
# speedup vs baseline: 1.3078x; 1.0756x over previous
"""Multi-head self-attention Trainium2 kernel.

Problem: B=2, N=2048, D=1024, H=16 heads (HD=64), fp32 I/O.

Sharding (8 cores): core c handles batch b = c//4 and the 4-head group
g = c%4 (data parallel on B, tensor parallel on heads).  Each core:
  1. QKV projection for its 768 columns (q cols pre-scaled by HD^-0.5,
     bias folded in as a K=1 matmul against a ones row), producing
     qT/kT channel-major and V row-major augmented with a ones column.
  2. Transposed attention, two heads packed per pass (head A in PE rows
     0-63, head B in rows 64-127 -> concurrent row-group matmuls):
     S^T[m, n] scores in PSUM, one exp per m-tile on ScalarE (no max
     subtraction -- logits are O(1) here), PV matmul contracting over m
     with the ones column yielding the softmax denominator as row 64.
  3. Normalization: fast-approx reciprocal of the denominator row,
     broadcast across 64 partitions via a K=1 matmul, multiply.
  4. Output projection against its 256 rows of w_proj -> fp32 partial.
Host sums the 4 partials per batch and adds b_proj.

Emission order is chosen to keep the PE array dense (HAM stays at
K=8/8): QKV matmul groups are injected one-per-iteration into the
first attention pair's ACT-bound window.
"""

import numpy as np
import ml_dtypes

B, N, D, H = 2, 2048, 1024, 16
HD = D // H  # 64
SCALE = HD ** -0.5
NCORES = 8
HPC = H // 4  # heads per core
CPC = HPC * HD  # channels per core = 256
P = 128
DT = D // P  # 8 contraction tiles
NT = N // P  # 16 sequence tiles

_CACHE = {}


def build_nc():
    import concourse.tile as tile
    from concourse import bacc, mybir

    nc = bacc.Bacc("TRN2", target_bir_lowering=False, debug=False,
                   num_devices=NCORES)
    bf16 = mybir.dt.bfloat16
    f32 = mybir.dt.float32
    xt = nc.dram_tensor("xt", [D, N], bf16, kind="ExternalInput").ap()
    w = nc.dram_tensor("w", [D + 1, 3 * CPC], bf16, kind="ExternalInput").ap()
    wp = nc.dram_tensor("wp", [CPC, D], bf16, kind="ExternalInput").ap()
    y = nc.dram_tensor("y", [N, D], f32, kind="ExternalOutput").ap()

    with tile.TileContext(nc) as tc:
        _mha_tile_kernel(tc, y, xt, w, wp)
    nc.compile()
    return nc


def _mha_tile_kernel(tc, y, xt, w, wp):
    from contextlib import ExitStack
    from concourse import mybir

    nc = tc.nc
    bf16 = mybir.dt.bfloat16
    f32 = mybir.dt.float32
    EXP = mybir.ActivationFunctionType.Exp

    with ExitStack() as ctx:
        consts = ctx.enter_context(tc.tile_pool(name="consts", bufs=1))
        work = ctx.enter_context(tc.tile_pool(name="work", bufs=1))
        ebpool = ctx.enter_context(tc.tile_pool(name="eb", bufs=3))
        ypool = ctx.enter_context(tc.tile_pool(name="yp", bufs=3))
        rpool = ctx.enter_context(tc.tile_pool(name="rp", bufs=2))
        ps_sc = ctx.enter_context(
            tc.tile_pool(name="ps_sc", bufs=2, space="PSUM"))   # 2x2 banks
        ps_pv = ctx.enter_context(
            tc.tile_pool(name="ps_pv", bufs=2, space="PSUM"))   # 2x1 banks
        ps_sm = ctx.enter_context(
            tc.tile_pool(name="ps_sm", bufs=1, space="PSUM"))   # 1x2 banks

        # ---- loads ----
        xt_sb = work.tile([P, DT, N], bf16, tag="xt")
        for kt in range(DT):
            nc.sync.dma_start(xt_sb[:, kt], xt[kt * P:(kt + 1) * P, :])
        w_sb = work.tile([P, DT, 3 * CPC], bf16, tag="w")
        for kt in range(DT):
            nc.sync.dma_start(w_sb[:, kt], w[kt * P:(kt + 1) * P, :])
        wb_sb = work.tile([1, 3 * CPC], bf16, tag="wb")
        nc.sync.dma_start(wb_sb, w[D:D + 1, :])
        wp_sb = work.tile([P, 2, D], bf16, tag="wp")
        for ct in range(2):
            nc.sync.dma_start(wp_sb[:, ct], wp[ct * P:(ct + 1) * P, :])
        ones_sb = consts.tile([1, N], bf16, tag="ones")
        nc.vector.memset(ones_sb, 1.0)

        qk_sb = work.tile([P, 4, N], bf16, tag="qk")
        vaug_sb = work.tile([P, NT, HPC, HD + 1], bf16, tag="vaug")
        nc.vector.memset(vaug_sb[:, :, :, HD:HD + 1], 1.0)
        outT_sb = work.tile([P, 2, N], bf16, tag="outT")

        # ---- emission helpers ----
        def emit_qk_group(ct, half, engine):
            """qT/kT channel-major: psum[c 128, n 1024] accumulated over
            d; bias via K=1 matmul; copy to qk_sb as bf16."""
            wcol = ct * P
            n0 = half * 1024
            ps = ps_sm.tile([P, 1024], f32, tag="sm", name=f"qk{ct}{half}")
            for dt in range(DT):
                for j in range(2):
                    nc.tensor.matmul(
                        ps[:, j * 512:(j + 1) * 512],
                        lhsT=w_sb[:, dt, wcol:wcol + P],
                        rhs=xt_sb[:, dt, n0 + j * 512:n0 + (j + 1) * 512],
                        start=(dt == 0), stop=False)
            for j in range(2):
                nc.tensor.matmul(
                    ps[:, j * 512:(j + 1) * 512],
                    lhsT=wb_sb[:, wcol:wcol + P],
                    rhs=ones_sb[:, n0 + j * 512:n0 + (j + 1) * 512],
                    start=False, stop=True)
            if engine is nc.scalar:
                nc.scalar.copy(out=qk_sb[:, ct, n0:n0 + 1024], in_=ps)
            else:
                engine.tensor_copy(out=qk_sb[:, ct, n0:n0 + 1024], in_=ps)

        def emit_v_group(mt, pair):
            """V row-major for head pair: psum[m 128, c 128] over d,
            bias via K=1, then per-head copies into vaug."""
            c0 = 2 * CPC + pair * P
            ps = ps_sm.tile([P, P], f32, tag="sm", name=f"v{pair}_{mt}")
            for dt in range(DT):
                nc.tensor.matmul(
                    ps, lhsT=xt_sb[:, dt, mt * P:(mt + 1) * P],
                    rhs=w_sb[:, dt, c0:c0 + P],
                    start=(dt == 0), stop=False)
            nc.tensor.matmul(
                ps, lhsT=ones_sb[:, mt * P:(mt + 1) * P],
                rhs=wb_sb[:, c0:c0 + P], start=False, stop=True)
            for i in range(2):
                nc.vector.tensor_copy(
                    out=vaug_sb[:, mt, 2 * pair + i, 0:HD],
                    in_=ps[:, i * HD:(i + 1) * HD])

        def emit_epilogue(pair, q, pv):
            """Normalize quarter q of head pair: reciprocal of denominator
            row, K=1 matmul broadcast, multiply into outT."""
            n0 = q * 512
            for i in range(2):
                bp = i * HD
                dcp = rpool.tile([1, 512], f32, tag="dcp")
                nc.vector.tensor_copy(out=dcp, in_=pv[i][HD:HD + 1, :])
                rec = rpool.tile([1, 512], f32, tag="rec")
                nc.vector.reciprocal_approx_fast(out=rec, in_=dcp)
                rbf = rpool.tile([1, 512], bf16, tag="rbf")
                nc.vector.tensor_copy(out=rbf, in_=rec)
                bc = ps_sm.tile([HD, 512], f32, tag="sm",
                                name=f"bc{pair}{q}{i}")
                nc.tensor.matmul(bc, lhsT=ones_sb[:, 0:HD], rhs=rbf,
                                 start=True, stop=True)
                bcs = rpool.tile([HD, 512], f32, tag="bcs")
                nc.vector.tensor_copy(out=bcs, in_=bc)
                nc.vector.tensor_mul(
                    out=outT_sb[bp:bp + HD, pair, n0:n0 + 512],
                    in0=pv[i][0:HD, :], in1=bcs)

        def emit_proj(nt):
            """Output projection rows nt*128..: one [128,1024] psum group
            (both 512-col halves), one copy, one DMA."""
            ps = ps_sm.tile([P, 1024], f32, tag="sm", name=f"pj{nt}")
            for ec in range(2):
                for ct in range(2):
                    nc.tensor.matmul(
                        ps[:, ec * 512:(ec + 1) * 512],
                        lhsT=outT_sb[:, ct, nt * P:(nt + 1) * P],
                        rhs=wp_sb[:, ct, ec * 512:(ec + 1) * 512],
                        start=(ct == 0), stop=(ct == 1))
            yt = ypool.tile([P, D], f32, tag="y")
            nc.vector.tensor_copy(out=yt, in_=ps)
            nc.sync.dma_start(y[nt * P:(nt + 1) * P, :], yt)

        # ---- prologue: just enough for attention pair 0 to start ----
        emit_qk_group(2, 0, nc.scalar)   # kT heads 0,1 first half of m
        emit_qk_group(0, 0, nc.scalar)   # qT heads 0,1 cols 0:1024
        for mt in range(4):
            emit_v_group(mt, 0)

        # work to interleave into attention pair 0 (one group/iteration)
        inject = [lambda: emit_qk_group(2, 1, nc.vector)]  # kT m 1024:2048
        inject += [lambda mt=mt: emit_v_group(mt, 0) for mt in range(4, NT)]
        inject.append(lambda: emit_qk_group(0, 1, nc.vector))
        inject.append(lambda: emit_qk_group(1, 0, nc.vector))
        inject.append(lambda: emit_qk_group(1, 1, nc.vector))
        inject.append(lambda: emit_qk_group(3, 0, nc.vector))
        inject.append(lambda: emit_qk_group(3, 1, nc.vector))
        inject += [lambda mt=mt: emit_v_group(mt, 1) for mt in range(NT)]

        # ---- attention: heads packed in pairs (rows 0-63 / 64-127) ----
        # Epilogues are emitted lazily (a few iterations into the NEXT
        # quarter) so their DVE chain never stalls the PE queue; proj
        # groups are injected into pair 1 once their outT columns exist.
        pending = []   # callables to emit a few iterations later
        for pair in range(2):
            for q in range(4):
                n0 = q * 512
                pv = [ps_pv.tile([HD + 1, 512], f32, tag="pv",
                                 name=f"pv{pair}{q}{i}") for i in range(2)]
                for mt in range(NT):
                    if pair == 0 and inject:
                        inject.pop(0)()
                    if mt == 4 and pending:
                        for fn in pending:
                            fn()
                        pending = []
                    ps = ps_sc.tile([P, 1024], f32, tag="sc")
                    for i in range(2):
                        bp = i * HD
                        nc.tensor.matmul(
                            ps[:, i * 512:(i + 1) * 512],
                            lhsT=qk_sb[bp:bp + HD, 2 + pair,
                                       mt * P:(mt + 1) * P],
                            rhs=qk_sb[bp:bp + HD, pair, n0:n0 + 512],
                            start=True, stop=True)
                    eb = ebpool.tile([P, 1024], bf16, tag="eb")
                    nc.scalar.activation(out=eb, in_=ps, func=EXP)
                    for i in range(2):
                        nc.tensor.matmul(
                            pv[i], lhsT=vaug_sb[:, mt, 2 * pair + i, :],
                            rhs=eb[:, i * 512:(i + 1) * 512],
                            start=(mt == 0), stop=(mt == NT - 1))
                pending.append(
                    lambda pair=pair, q=q, pv=pv: emit_epilogue(pair, q, pv))
                if pair == 1:
                    # proj rows for quarter q-1 (outT cols complete after
                    # the lazily-emitted epilogues of both pairs)
                    if q >= 1:
                        pending += [lambda nt=nt: emit_proj(nt)
                                    for nt in range(4 * (q - 1), 4 * q)]
        for fn in pending:
            fn()
        for nt in range(12, NT):
            emit_proj(nt)


def make_in_maps(x, w_qkv, b_qkv, w_proj):
    """Build the 8 per-core input dicts (host-side sharding)."""
    bf = ml_dtypes.bfloat16
    x = np.asarray(x, np.float32)
    w_qkv = np.asarray(w_qkv, np.float32)
    b_qkv = np.asarray(b_qkv, np.float32)
    w_proj = np.asarray(w_proj, np.float32)

    xts = [np.ascontiguousarray(x[b].T).astype(bf) for b in range(B)]
    w_augs = []
    wps = []
    for g in range(4):
        c0 = g * CPC
        wq = w_qkv[:, c0:c0 + CPC] * SCALE
        wk = w_qkv[:, D + c0:D + c0 + CPC]
        wv = w_qkv[:, 2 * D + c0:2 * D + c0 + CPC]
        bq = b_qkv[c0:c0 + CPC] * SCALE
        bk = b_qkv[D + c0:D + c0 + CPC]
        bv = b_qkv[2 * D + c0:2 * D + c0 + CPC]
        w_slice = np.concatenate([wq, wk, wv], axis=1)
        b_slice = np.concatenate([bq, bk, bv])
        w_aug = np.vstack([w_slice, b_slice[None, :]]).astype(bf)
        w_augs.append(np.ascontiguousarray(w_aug))
        wps.append(np.ascontiguousarray(w_proj[c0:c0 + CPC, :]).astype(bf))

    in_maps = []
    for core in range(NCORES):
        b, g = core // 4, core % 4
        in_maps.append({"xt": xts[b], "w": w_augs[g], "wp": wps[g]})
    return in_maps


def core_reference(in_map):
    """Numpy reference for ONE core's shard (for CoreSim verification)."""
    xt = np.asarray(in_map["xt"], np.float32)  # [D, N]
    w = np.asarray(in_map["w"], np.float32)    # [D+1, 768]
    wp = np.asarray(in_map["wp"], np.float32)  # [256, D]
    qkv = xt.T @ w[:D] + w[D]                  # [N, 768]
    out = np.zeros((N, CPC), np.float32)
    for h in range(HPC):
        q = qkv[:, h * HD:(h + 1) * HD]
        k = qkv[:, CPC + h * HD:CPC + (h + 1) * HD]
        v = qkv[:, 2 * CPC + h * HD:2 * CPC + (h + 1) * HD]
        s = q @ k.T  # scale already folded into wq
        p = np.exp(s - s.max(axis=-1, keepdims=True))
        p /= p.sum(axis=-1, keepdims=True)
        out[:, h * HD:(h + 1) * HD] = p @ v
    return out @ wp  # [N, D] partial


def kernel(x, w_qkv, b_qkv, w_proj, b_proj):
    from concourse.bass_utils import run_bass_kernel_spmd

    in_maps = make_in_maps(x, w_qkv, b_qkv, w_proj)
    if "nc" not in _CACHE:
        _CACHE["nc"] = build_nc()
    res = run_bass_kernel_spmd(_CACHE["nc"], in_maps,
                               core_ids=list(range(NCORES)))
    outs = [r["y"] for r in res.results]
    y = np.empty((B, N, D), np.float32)
    for b in range(B):
        y[b] = outs[4 * b] + outs[4 * b + 1] + outs[4 * b + 2] + outs[4 * b + 3]
    y += np.asarray(b_proj, np.float32)
    return y


# revision 15
# speedup vs baseline: 1.5254x; 1.1664x over previous
"""Multi-head self-attention Trainium2 kernel.

Problem: B=2, N=2048, D=1024, H=16 heads (HD=64), fp32 I/O.

Sharding (8 cores): core c handles batch b = c//4 and the 4-head group
g = c%4 (data parallel on B, tensor parallel on heads).  Each core:
  1. QKV projection for its 768 columns (q cols pre-scaled by HD^-0.5,
     bias folded in as a K=1 matmul against a ones row), producing
     qT/kT channel-major and V row-major augmented with a ones column.
  2. Transposed attention, two heads packed per pass (head A in PE rows
     0-63, head B in rows 64-127 -> concurrent row-group matmuls):
     S^T[m, n] scores in PSUM, one exp per m-tile on ScalarE (no max
     subtraction -- logits are O(1) here), PV matmul contracting over m
     with the ones column yielding the softmax denominator as row 64.
  3. Normalization: fast-approx reciprocal of the denominator row,
     broadcast across 64 partitions via a K=1 matmul, multiply.
  4. Output projection against its 256 rows of w_proj -> fp32 partial.
Host sums the 4 partials per batch and adds b_proj.

Emission order is chosen to keep the PE array dense (HAM stays at
K=8/8): QKV matmul groups are injected one-per-iteration into the
first attention pair's ACT-bound window.
"""

import numpy as np
import ml_dtypes

B, N, D, H = 2, 2048, 1024, 16
HD = D // H  # 64
SCALE = HD ** -0.5
NCORES = 8
HPC = H // 4  # heads per core
CPC = HPC * HD  # channels per core = 256
P = 128
DT = D // P  # 8 contraction tiles
NT = N // P  # 16 sequence tiles

_CACHE = {}


def build_nc():
    import concourse.tile as tile
    from concourse import bacc, mybir

    nc = bacc.Bacc("TRN2", target_bir_lowering=False, debug=False,
                   num_devices=NCORES)
    bf16 = mybir.dt.bfloat16
    f32 = mybir.dt.float32
    xt = nc.dram_tensor("xt", [D, N], bf16, kind="ExternalInput").ap()
    w = nc.dram_tensor("w", [D + 1, 3 * CPC], bf16, kind="ExternalInput").ap()
    wp = nc.dram_tensor("wp", [CPC, D], bf16, kind="ExternalInput").ap()
    y = nc.dram_tensor("y", [N, D], f32, kind="ExternalOutput").ap()

    with tile.TileContext(nc) as tc:
        _mha_tile_kernel(tc, y, xt, w, wp)
    nc.compile()
    return nc


def _mha_tile_kernel(tc, y, xt, w, wp):
    from contextlib import ExitStack
    from concourse import mybir

    nc = tc.nc
    bf16 = mybir.dt.bfloat16
    f32 = mybir.dt.float32
    EXP = mybir.ActivationFunctionType.Exp

    with ExitStack() as ctx:
        consts = ctx.enter_context(tc.tile_pool(name="consts", bufs=1))
        work = ctx.enter_context(tc.tile_pool(name="work", bufs=1))
        ebpool = ctx.enter_context(tc.tile_pool(name="eb", bufs=3))
        ypool = ctx.enter_context(tc.tile_pool(name="yp", bufs=3))
        rpool = ctx.enter_context(tc.tile_pool(name="rp", bufs=2))
        pvspool = ctx.enter_context(tc.tile_pool(name="pvs", bufs=4))
        ps_sc = ctx.enter_context(
            tc.tile_pool(name="ps_sc", bufs=2, space="PSUM"))   # 2x2 banks
        ps_pv = ctx.enter_context(
            tc.tile_pool(name="ps_pv", bufs=2, space="PSUM"))   # 2x1 banks
        ps_sm = ctx.enter_context(
            tc.tile_pool(name="ps_sm", bufs=1, space="PSUM"))   # 1x2 banks

        # ---- loads ----
        xt_sb = work.tile([P, DT, N], bf16, tag="xt")
        for kt in range(DT):
            nc.sync.dma_start(xt_sb[:, kt], xt[kt * P:(kt + 1) * P, :])
        w_sb = work.tile([P, DT, 3 * CPC], bf16, tag="w")
        for kt in range(DT):
            nc.sync.dma_start(w_sb[:, kt], w[kt * P:(kt + 1) * P, :])
        wb_sb = work.tile([1, 3 * CPC], bf16, tag="wb")
        nc.sync.dma_start(wb_sb, w[D:D + 1, :])
        wp_sb = work.tile([P, 2, D], bf16, tag="wp")
        for ct in range(2):
            nc.sync.dma_start(wp_sb[:, ct], wp[ct * P:(ct + 1) * P, :])
        ones_sb = consts.tile([1, N], bf16, tag="ones")
        nc.vector.memset(ones_sb, 1.0)

        qk_sb = work.tile([P, 4, N], bf16, tag="qk")
        vaug_sb = work.tile([P, NT, HPC, HD + 1], bf16, tag="vaug")
        nc.vector.memset(vaug_sb[:, :, :, HD:HD + 1], 1.0)
        outT_sb = work.tile([P, 2, N], bf16, tag="outT")

        # ---- emission helpers ----
        def emit_qk_group(ct, half, engine):
            """qT/kT channel-major: psum[c 128, n 1024] accumulated over
            d; bias via K=1 matmul; copy to qk_sb as bf16."""
            wcol = ct * P
            n0 = half * 1024
            ps = ps_sm.tile([P, 1024], f32, tag="sm", name=f"qk{ct}{half}")
            for dt in range(DT):
                for j in range(2):
                    nc.tensor.matmul(
                        ps[:, j * 512:(j + 1) * 512],
                        lhsT=w_sb[:, dt, wcol:wcol + P],
                        rhs=xt_sb[:, dt, n0 + j * 512:n0 + (j + 1) * 512],
                        start=(dt == 0), stop=False)
            for j in range(2):
                nc.tensor.matmul(
                    ps[:, j * 512:(j + 1) * 512],
                    lhsT=wb_sb[:, wcol:wcol + P],
                    rhs=ones_sb[:, n0 + j * 512:n0 + (j + 1) * 512],
                    start=False, stop=True)
            if engine is nc.scalar:
                nc.scalar.copy(out=qk_sb[:, ct, n0:n0 + 1024], in_=ps)
            else:
                engine.tensor_copy(out=qk_sb[:, ct, n0:n0 + 1024], in_=ps)

        def emit_v_group(mt):
            """V row-major, all 4 heads: psum[m 128, c 256] over d,
            bias via K=1, then per-head copies into vaug."""
            c0 = 2 * CPC
            ps = ps_sm.tile([P, CPC], f32, tag="sm", name=f"v{mt}")
            for dt in range(DT):
                nc.tensor.matmul(
                    ps, lhsT=xt_sb[:, dt, mt * P:(mt + 1) * P],
                    rhs=w_sb[:, dt, c0:c0 + CPC],
                    start=(dt == 0), stop=False)
            nc.tensor.matmul(
                ps, lhsT=ones_sb[:, mt * P:(mt + 1) * P],
                rhs=wb_sb[:, c0:c0 + CPC], start=False, stop=True)
            for i in range(HPC):
                nc.vector.tensor_copy(
                    out=vaug_sb[:, mt, i, 0:HD],
                    in_=ps[:, i * HD:(i + 1) * HD])

        def emit_pv_release(pvs, pv):
            """Copy PV psum accumulators to SBUF right at quarter end so
            the psum banks free fast (next quarter's PV never stalls)."""
            for i in range(2):
                nc.vector.tensor_copy(out=pvs[i], in_=pv[i])

        def emit_epilogue(pair, q, pvs):
            """Normalize quarter q of head pair from the SBUF copy:
            fast reciprocal of denominator row, K=1 matmul broadcast,
            multiply into outT."""
            n0 = q * 512
            for i in range(2):
                bp = i * HD
                dcp = rpool.tile([1, 512], f32, tag="dcp")
                nc.vector.tensor_copy(out=dcp, in_=pvs[i][HD:HD + 1, :])
                rec = rpool.tile([1, 512], f32, tag="rec")
                nc.vector.reciprocal_approx_fast(out=rec, in_=dcp)
                rbf = rpool.tile([1, 512], bf16, tag="rbf")
                nc.vector.tensor_copy(out=rbf, in_=rec)
                bc = ps_sm.tile([HD, 512], f32, tag="sm",
                                name=f"bc{pair}{q}{i}")
                nc.tensor.matmul(bc, lhsT=ones_sb[:, 0:HD], rhs=rbf,
                                 start=True, stop=True)
                nc.vector.tensor_mul(
                    out=outT_sb[bp:bp + HD, pair, n0:n0 + 512],
                    in0=bc, in1=pvs[i][0:HD, :])

        def emit_proj(nt):
            """Output projection rows nt*128..: one [128,1024] psum group
            (ct outer so consecutive matmuls share weights), one copy,
            one DMA."""
            ps = ps_sm.tile([P, 1024], f32, tag="sm", name=f"pj{nt}")
            for ct in range(2):
                for ec in range(2):
                    nc.tensor.matmul(
                        ps[:, ec * 512:(ec + 1) * 512],
                        lhsT=outT_sb[:, ct, nt * P:(nt + 1) * P],
                        rhs=wp_sb[:, ct, ec * 512:(ec + 1) * 512],
                        start=(ct == 0), stop=(ct == 1))
            yt = ypool.tile([P, D], f32, tag="y")
            nc.vector.tensor_copy(out=yt, in_=ps)
            nc.sync.dma_start(y[nt * P:(nt + 1) * P, :], yt)

        # ---- prologue: just enough for attention pair 0 to start ----
        emit_qk_group(2, 0, nc.scalar)   # kT heads 0,1 first half of m
        emit_qk_group(0, 0, nc.scalar)   # qT heads 0,1 cols 0:1024
        for mt in range(4):
            emit_v_group(mt)

        # work to interleave into attention pair 0 (one group/iteration)
        inject = [lambda: emit_qk_group(2, 1, nc.vector)]  # kT m 1024:2048
        inject += [lambda mt=mt: emit_v_group(mt) for mt in range(4, NT)]
        inject.append(lambda: emit_qk_group(0, 1, nc.vector))
        inject.append(lambda: emit_qk_group(1, 0, nc.vector))
        inject.append(lambda: emit_qk_group(1, 1, nc.vector))
        inject.append(lambda: emit_qk_group(3, 0, nc.vector))
        inject.append(lambda: emit_qk_group(3, 1, nc.vector))

        # ---- attention: heads packed in pairs (rows 0-63 / 64-127) ----
        # PV accumulators are copied to SBUF right at quarter end (psum
        # frees fast); the normalize/proj chains are emitted lazily a few
        # iterations into the NEXT quarter so they never stall the PE.
        pending = []   # callables to emit a few iterations later
        for pair in range(2):
            for q in range(4):
                n0 = q * 512
                pv = [ps_pv.tile([HD + 1, 512], f32, tag="pv",
                                 name=f"pv{pair}{q}{i}") for i in range(2)]
                for mt in range(NT):
                    if pair == 0 and inject:
                        inject.pop(0)()
                    if mt == 4 and pending:
                        for fn in pending:
                            fn()
                        pending = []
                    ps = ps_sc.tile([P, 1024], f32, tag="sc")
                    for i in range(2):
                        bp = i * HD
                        nc.tensor.matmul(
                            ps[:, i * 512:(i + 1) * 512],
                            lhsT=qk_sb[bp:bp + HD, 2 + pair,
                                       mt * P:(mt + 1) * P],
                            rhs=qk_sb[bp:bp + HD, pair, n0:n0 + 512],
                            start=True, stop=True)
                    eb = ebpool.tile([P, 1024], bf16, tag="eb")
                    nc.scalar.activation(out=eb, in_=ps, func=EXP)
                    for i in range(2):
                        nc.tensor.matmul(
                            pv[i], lhsT=vaug_sb[:, mt, 2 * pair + i, :],
                            rhs=eb[:, i * 512:(i + 1) * 512],
                            start=(mt == 0), stop=(mt == NT - 1))
                pvs = [pvspool.tile([HD + 1, 512], f32, tag="pvs",
                                    name=f"pvs{pair}{q}{i}")
                       for i in range(2)]
                emit_pv_release(pvs, pv)
                pending.append(
                    lambda pair=pair, q=q, pvs=pvs:
                    emit_epilogue(pair, q, pvs))
                if pair == 1:
                    pending += [lambda nt=nt: emit_proj(nt)
                                for nt in range(4 * q, 4 * q + 4)]
        for fn in pending:
            fn()


def make_in_maps(x, w_qkv, b_qkv, w_proj):
    """Build the 8 per-core input dicts (host-side sharding)."""
    bf = ml_dtypes.bfloat16
    x = np.asarray(x, np.float32)
    w_qkv = np.asarray(w_qkv, np.float32)
    b_qkv = np.asarray(b_qkv, np.float32)
    w_proj = np.asarray(w_proj, np.float32)

    xts = [np.ascontiguousarray(x[b].T).astype(bf) for b in range(B)]
    w_augs = []
    wps = []
    for g in range(4):
        c0 = g * CPC
        wq = w_qkv[:, c0:c0 + CPC] * SCALE
        wk = w_qkv[:, D + c0:D + c0 + CPC]
        wv = w_qkv[:, 2 * D + c0:2 * D + c0 + CPC]
        bq = b_qkv[c0:c0 + CPC] * SCALE
        bk = b_qkv[D + c0:D + c0 + CPC]
        bv = b_qkv[2 * D + c0:2 * D + c0 + CPC]
        w_slice = np.concatenate([wq, wk, wv], axis=1)
        b_slice = np.concatenate([bq, bk, bv])
        w_aug = np.vstack([w_slice, b_slice[None, :]]).astype(bf)
        w_augs.append(np.ascontiguousarray(w_aug))
        wps.append(np.ascontiguousarray(w_proj[c0:c0 + CPC, :]).astype(bf))

    in_maps = []
    for core in range(NCORES):
        b, g = core // 4, core % 4
        in_maps.append({"xt": xts[b], "w": w_augs[g], "wp": wps[g]})
    return in_maps


def core_reference(in_map):
    """Numpy reference for ONE core's shard (for CoreSim verification)."""
    xt = np.asarray(in_map["xt"], np.float32)  # [D, N]
    w = np.asarray(in_map["w"], np.float32)    # [D+1, 768]
    wp = np.asarray(in_map["wp"], np.float32)  # [256, D]
    qkv = xt.T @ w[:D] + w[D]                  # [N, 768]
    out = np.zeros((N, CPC), np.float32)
    for h in range(HPC):
        q = qkv[:, h * HD:(h + 1) * HD]
        k = qkv[:, CPC + h * HD:CPC + (h + 1) * HD]
        v = qkv[:, 2 * CPC + h * HD:2 * CPC + (h + 1) * HD]
        s = q @ k.T  # scale already folded into wq
        p = np.exp(s - s.max(axis=-1, keepdims=True))
        p /= p.sum(axis=-1, keepdims=True)
        out[:, h * HD:(h + 1) * HD] = p @ v
    return out @ wp  # [N, D] partial


def kernel(x, w_qkv, b_qkv, w_proj, b_proj):
    from concourse.bass_utils import run_bass_kernel_spmd

    in_maps = make_in_maps(x, w_qkv, b_qkv, w_proj)
    if "nc" not in _CACHE:
        _CACHE["nc"] = build_nc()
    res = run_bass_kernel_spmd(_CACHE["nc"], in_maps,
                               core_ids=list(range(NCORES)))
    outs = [r["y"] for r in res.results]
    y = np.empty((B, N, D), np.float32)
    for b in range(B):
        y[b] = outs[4 * b] + outs[4 * b + 1] + outs[4 * b + 2] + outs[4 * b + 3]
    y += np.asarray(b_proj, np.float32)
    return y


# revision 20
# speedup vs baseline: 1.5547x; 1.0192x over previous
"""Multi-head self-attention Trainium2 kernel.

Problem: B=2, N=2048, D=1024, H=16 heads (HD=64), fp32 I/O.

Sharding (8 cores): core c handles batch b = c//4 and the 4-head group
g = c%4 (data parallel on B, tensor parallel on heads).  Each core:
  1. QKV projection for its 768 columns (q cols pre-scaled by HD^-0.5,
     bias folded in as a K=1 matmul against a ones row), producing
     qT/kT channel-major and V row-major augmented with a ones column.
  2. Transposed attention, two heads packed per pass (head A in PE rows
     0-63, head B in rows 64-127 -> concurrent row-group matmuls):
     S^T[m, n] scores in PSUM, one exp per m-tile on ScalarE (no max
     subtraction -- logits are O(1) here), PV matmul contracting over m
     with the ones column yielding the softmax denominator as row 64.
  3. Normalization: fast-approx reciprocal of the denominator row,
     broadcast across 64 partitions via a K=1 matmul, multiply.
  4. Output projection against its 256 rows of w_proj -> fp32 partial.
Host sums the 4 partials per batch and adds b_proj.

Emission order is chosen to keep the PE array dense (HAM stays at
K=8/8): QKV matmul groups are injected one-per-iteration into the
first attention pair's ACT-bound window.
"""

import numpy as np
import ml_dtypes

B, N, D, H = 2, 2048, 1024, 16
HD = D // H  # 64
SCALE = HD ** -0.5
NCORES = 8
HPC = H // 4  # heads per core
CPC = HPC * HD  # channels per core = 256
P = 128
DT = D // P  # 8 contraction tiles
NT = N // P  # 16 sequence tiles

_CACHE = {}


def build_nc():
    import concourse.tile as tile
    from concourse import bacc, mybir

    nc = bacc.Bacc("TRN2", target_bir_lowering=False, debug=False,
                   num_devices=NCORES)
    bf16 = mybir.dt.bfloat16
    f32 = mybir.dt.float32
    xt = nc.dram_tensor("xt", [D, N], bf16, kind="ExternalInput").ap()
    w = nc.dram_tensor("w", [D + 1, 3 * CPC], bf16, kind="ExternalInput").ap()
    wp = nc.dram_tensor("wp", [CPC, D], bf16, kind="ExternalInput").ap()
    y = nc.dram_tensor("y", [N, D], f32, kind="ExternalOutput").ap()

    with tile.TileContext(nc) as tc:
        _mha_tile_kernel(tc, y, xt, w, wp)
    nc.compile()
    return nc


def _mha_tile_kernel(tc, y, xt, w, wp):
    from contextlib import ExitStack
    from concourse import mybir

    nc = tc.nc
    bf16 = mybir.dt.bfloat16
    f32 = mybir.dt.float32
    EXP = mybir.ActivationFunctionType.Exp

    with ExitStack() as ctx:
        consts = ctx.enter_context(tc.tile_pool(name="consts", bufs=1))
        work = ctx.enter_context(tc.tile_pool(name="work", bufs=1))
        ebpool = ctx.enter_context(tc.tile_pool(name="eb", bufs=3))
        ypool = ctx.enter_context(tc.tile_pool(name="yp", bufs=3))
        rpool = ctx.enter_context(tc.tile_pool(name="rp", bufs=2))
        pvspool = ctx.enter_context(tc.tile_pool(name="pvs", bufs=4))
        ps_sc = ctx.enter_context(
            tc.tile_pool(name="ps_sc", bufs=2, space="PSUM"))   # 2x2 banks
        ps_pv = ctx.enter_context(
            tc.tile_pool(name="ps_pv", bufs=2, space="PSUM"))   # 2x1 banks
        ps_sm = ctx.enter_context(
            tc.tile_pool(name="ps_sm", bufs=1, space="PSUM"))   # 1x2 banks

        # ---- loads (w first; xt column-half 0 before column-half 1 so
        # the first qk/v groups and attention quarter 0 start early) ----
        w_sb = work.tile([P, DT, 3 * CPC], bf16, tag="w")
        for kt in range(DT):
            nc.sync.dma_start(w_sb[:, kt], w[kt * P:(kt + 1) * P, :])
        xt_sb = work.tile([P, DT, N], bf16, tag="xt")
        for half in range(2):
            for kt in range(DT):
                nc.sync.dma_start(
                    xt_sb[:, kt, half * 1024:(half + 1) * 1024],
                    xt[kt * P:(kt + 1) * P, half * 1024:(half + 1) * 1024])
        wb_sb = work.tile([1, 3 * CPC], bf16, tag="wb")
        nc.sync.dma_start(wb_sb, w[D:D + 1, :])
        wp_sb = work.tile([P, 2, D], bf16, tag="wp")
        for ct in range(2):
            nc.sync.dma_start(wp_sb[:, ct], wp[ct * P:(ct + 1) * P, :])
        ones_sb = consts.tile([1, N], bf16, tag="ones")
        nc.vector.memset(ones_sb, 1.0)

        qk_sb = work.tile([P, 4, N], bf16, tag="qk")
        vaug_sb = work.tile([P, NT, HPC, HD + 1], bf16, tag="vaug")
        nc.vector.memset(vaug_sb[:, :, :, HD:HD + 1], 1.0)
        outT_sb = work.tile([P, 2, N], bf16, tag="outT")

        # ---- emission helpers ----
        def qk_group_chunks(ct, half):
            """qT/kT channel-major: psum[c 128, n 1024] accumulated over
            d; bias via K=1 matmul; copy to qk_sb as bf16.  Returned as
            two ~2us emission chunks so injections never starve ACT."""
            wcol = ct * P
            n0 = half * 1024
            state = {}

            def emit_dts(dts, last):
                if not state:
                    state["ps"] = ps_sm.tile([P, 1024], f32, tag="sm",
                                             name=f"qk{ct}{half}")
                ps = state["ps"]
                for dt in dts:
                    for j in range(2):
                        nc.tensor.matmul(
                            ps[:, j * 512:(j + 1) * 512],
                            lhsT=w_sb[:, dt, wcol:wcol + P],
                            rhs=xt_sb[:, dt,
                                      n0 + j * 512:n0 + (j + 1) * 512],
                            start=(dt == 0), stop=False)
                if last:
                    for j in range(2):
                        nc.tensor.matmul(
                            ps[:, j * 512:(j + 1) * 512],
                            lhsT=wb_sb[:, wcol:wcol + P],
                            rhs=ones_sb[:, n0 + j * 512:n0 + (j + 1) * 512],
                            start=False, stop=True)
                    nc.vector.tensor_copy(out=qk_sb[:, ct, n0:n0 + 1024],
                                          in_=ps)

            return [lambda: emit_dts(range(0, 4), False),
                    lambda: emit_dts(range(4, DT), True)]

        def emit_qk_group(ct, half):
            for fn in qk_group_chunks(ct, half):
                fn()

        def emit_v_group(mt):
            """V row-major, all 4 heads: psum[m 128, c 256] over d,
            bias via K=1, then per-head copies into vaug."""
            c0 = 2 * CPC
            ps = ps_sm.tile([P, CPC], f32, tag="sm", name=f"v{mt}")
            for dt in range(DT):
                nc.tensor.matmul(
                    ps, lhsT=xt_sb[:, dt, mt * P:(mt + 1) * P],
                    rhs=w_sb[:, dt, c0:c0 + CPC],
                    start=(dt == 0), stop=False)
            nc.tensor.matmul(
                ps, lhsT=ones_sb[:, mt * P:(mt + 1) * P],
                rhs=wb_sb[:, c0:c0 + CPC], start=False, stop=True)
            for i in range(HPC):
                nc.vector.tensor_copy(
                    out=vaug_sb[:, mt, i, 0:HD],
                    in_=ps[:, i * HD:(i + 1) * HD])

        def emit_pv_release(pvs, pv):
            """Copy PV psum accumulators to SBUF right at quarter end so
            the psum banks free fast (next quarter's PV never stalls)."""
            for i in range(2):
                nc.vector.tensor_copy(out=pvs[i], in_=pv[i])

        def emit_epilogue(pair, q, pvs):
            """Normalize quarter q of head pair from the SBUF copy:
            fast reciprocal of denominator row, K=1 matmul broadcast,
            multiply into outT."""
            n0 = q * 512
            for i in range(2):
                bp = i * HD
                dcp = rpool.tile([1, 512], f32, tag="dcp")
                nc.vector.tensor_copy(out=dcp, in_=pvs[i][HD:HD + 1, :])
                rec = rpool.tile([1, 512], f32, tag="rec")
                nc.vector.reciprocal_approx_fast(out=rec, in_=dcp)
                rbf = rpool.tile([1, 512], bf16, tag="rbf")
                nc.vector.tensor_copy(out=rbf, in_=rec)
                bc = ps_sm.tile([HD, 512], f32, tag="sm",
                                name=f"bc{pair}{q}{i}")
                nc.tensor.matmul(bc, lhsT=ones_sb[:, 0:HD], rhs=rbf,
                                 start=True, stop=True)
                nc.vector.tensor_mul(
                    out=outT_sb[bp:bp + HD, pair, n0:n0 + 512],
                    in0=bc, in1=pvs[i][0:HD, :])

        def emit_proj(nt, tail=False):
            """Output projection rows nt*128..: one [128,1024] psum group
            (ct outer so consecutive matmuls share weights), one copy,
            one DMA.  Tail groups use the (by then idle) scores pool for
            double buffering."""
            pool = ps_sc if tail else ps_sm
            ps = pool.tile([P, 1024], f32, tag="sc" if tail else "sm",
                           name=f"pj{nt}")
            for ct in range(2):
                for ec in range(2):
                    nc.tensor.matmul(
                        ps[:, ec * 512:(ec + 1) * 512],
                        lhsT=outT_sb[:, ct, nt * P:(nt + 1) * P],
                        rhs=wp_sb[:, ct, ec * 512:(ec + 1) * 512],
                        start=(ct == 0), stop=(ct == 1))
            yt = ypool.tile([P, D], f32, tag="y")
            nc.vector.tensor_copy(out=yt, in_=ps)
            nc.sync.dma_start(y[nt * P:(nt + 1) * P, :], yt)

        # ---- prologue: just enough for attention pair 0 to start ----
        emit_qk_group(2, 0)   # kT heads 0,1 first half of m
        emit_qk_group(0, 0)   # qT heads 0,1 cols 0:1024
        for mt in range(4):
            emit_v_group(mt)

        # work to interleave into attention pair 0 (one chunk/iteration)
        inject = qk_group_chunks(2, 1)   # kT m 1024:2048
        inject += [lambda mt=mt: emit_v_group(mt) for mt in range(4, NT)]
        inject += qk_group_chunks(0, 1)
        inject += qk_group_chunks(1, 0)
        inject += qk_group_chunks(1, 1)
        inject += qk_group_chunks(3, 0)
        inject += qk_group_chunks(3, 1)

        # ---- attention: heads packed in pairs (rows 0-63 / 64-127) ----
        # PV accumulators are copied to SBUF right at quarter end (psum
        # frees fast); the normalize/proj chains are emitted lazily a few
        # iterations into the NEXT quarter so they never stall the PE.
        pending = []   # callables to emit a few iterations later
        for pair in range(2):
            for q in range(4):
                n0 = q * 512
                pv = [ps_pv.tile([HD + 1, 512], f32, tag="pv",
                                 name=f"pv{pair}{q}{i}") for i in range(2)]
                for mt in range(NT):
                    if pair == 0 and inject:
                        inject.pop(0)()
                    if mt == 4 and pending:
                        for fn in pending:
                            fn()
                        pending = []
                    ps = ps_sc.tile([P, 1024], f32, tag="sc")
                    for i in range(2):
                        bp = i * HD
                        nc.tensor.matmul(
                            ps[:, i * 512:(i + 1) * 512],
                            lhsT=qk_sb[bp:bp + HD, 2 + pair,
                                       mt * P:(mt + 1) * P],
                            rhs=qk_sb[bp:bp + HD, pair, n0:n0 + 512],
                            start=True, stop=True)
                    eb = ebpool.tile([P, 1024], bf16, tag="eb")
                    nc.scalar.activation(out=eb, in_=ps, func=EXP)
                    for i in range(2):
                        nc.tensor.matmul(
                            pv[i], lhsT=vaug_sb[:, mt, 2 * pair + i, :],
                            rhs=eb[:, i * 512:(i + 1) * 512],
                            start=(mt == 0), stop=(mt == NT - 1))
                pvs = [pvspool.tile([HD + 1, 512], f32, tag="pvs",
                                    name=f"pvs{pair}{q}{i}")
                       for i in range(2)]
                emit_pv_release(pvs, pv)
                pending.append(
                    lambda pair=pair, q=q, pvs=pvs:
                    emit_epilogue(pair, q, pvs))
                if pair == 1:
                    pending += [lambda nt=nt, t=(q == 3): emit_proj(nt, t)
                                for nt in range(4 * q, 4 * q + 4)]
        for fn in pending:
            fn()


def make_in_maps(x, w_qkv, b_qkv, w_proj):
    """Build the 8 per-core input dicts (host-side sharding)."""
    bf = ml_dtypes.bfloat16
    x = np.asarray(x, np.float32)
    w_qkv = np.asarray(w_qkv, np.float32)
    b_qkv = np.asarray(b_qkv, np.float32)
    w_proj = np.asarray(w_proj, np.float32)

    xts = [np.ascontiguousarray(x[b].T).astype(bf) for b in range(B)]
    w_augs = []
    wps = []
    for g in range(4):
        c0 = g * CPC
        wq = w_qkv[:, c0:c0 + CPC] * SCALE
        wk = w_qkv[:, D + c0:D + c0 + CPC]
        wv = w_qkv[:, 2 * D + c0:2 * D + c0 + CPC]
        bq = b_qkv[c0:c0 + CPC] * SCALE
        bk = b_qkv[D + c0:D + c0 + CPC]
        bv = b_qkv[2 * D + c0:2 * D + c0 + CPC]
        w_slice = np.concatenate([wq, wk, wv], axis=1)
        b_slice = np.concatenate([bq, bk, bv])
        w_aug = np.vstack([w_slice, b_slice[None, :]]).astype(bf)
        w_augs.append(np.ascontiguousarray(w_aug))
        wps.append(np.ascontiguousarray(w_proj[c0:c0 + CPC, :]).astype(bf))

    in_maps = []
    for core in range(NCORES):
        b, g = core // 4, core % 4
        in_maps.append({"xt": xts[b], "w": w_augs[g], "wp": wps[g]})
    return in_maps


def core_reference(in_map):
    """Numpy reference for ONE core's shard (for CoreSim verification)."""
    xt = np.asarray(in_map["xt"], np.float32)  # [D, N]
    w = np.asarray(in_map["w"], np.float32)    # [D+1, 768]
    wp = np.asarray(in_map["wp"], np.float32)  # [256, D]
    qkv = xt.T @ w[:D] + w[D]                  # [N, 768]
    out = np.zeros((N, CPC), np.float32)
    for h in range(HPC):
        q = qkv[:, h * HD:(h + 1) * HD]
        k = qkv[:, CPC + h * HD:CPC + (h + 1) * HD]
        v = qkv[:, 2 * CPC + h * HD:2 * CPC + (h + 1) * HD]
        s = q @ k.T  # scale already folded into wq
        p = np.exp(s - s.max(axis=-1, keepdims=True))
        p /= p.sum(axis=-1, keepdims=True)
        out[:, h * HD:(h + 1) * HD] = p @ v
    return out @ wp  # [N, D] partial


def kernel(x, w_qkv, b_qkv, w_proj, b_proj):
    from concourse.bass_utils import run_bass_kernel_spmd

    in_maps = make_in_maps(x, w_qkv, b_qkv, w_proj)
    if "nc" not in _CACHE:
        _CACHE["nc"] = build_nc()
    res = run_bass_kernel_spmd(_CACHE["nc"], in_maps,
                               core_ids=list(range(NCORES)))
    outs = [r["y"] for r in res.results]
    y = np.empty((B, N, D), np.float32)
    for b in range(B):
        y[b] = outs[4 * b] + outs[4 * b + 1] + outs[4 * b + 2] + outs[4 * b + 3]
    y += np.asarray(b_proj, np.float32)
    return y
